# revision 10
# baseline (speedup 1.0000x reference)
"""Trainium2 Bass kernel for nn_CausalSelfAttention (B=4, T=2048, C=1024, H=1).

Sharding: 8 cores = 4 batches x 2 causal-balanced query folds (1024 queries
each).  No collectives.  Each core computes the full pipeline for its query
set: q/k/v projections, causal softmax attention, and the output projection.

Dataflow is transpose-free on device:
  - host supplies q^T, k^T, v^T and W^T (cin on the partition axis)
  - projections emit qp^T [d, tq] and kp^T [d, tk] (d on partitions) and
    vp [tk, d] (tk on partitions)
  - scores are computed transposed, S^T[tk, tq], so the exp() output feeds
    the PV matmul directly as the moving operand (no on-chip transposes)
  - softmax denominators come from a ones-row matmul; normalization is folded
    into the final-projection eviction (row scaling commutes with x @ Wff^T)

Causality: 4 query slots of 256 per core with fixed tk-tile budgets
[16, 12, 8, 4].  Fold 0 takes query blocks at offsets [1792, 1280, 512, 0],
fold 1 takes [1536, 1024, 768, 256]; both need <= the same budgets, so one
SPMD program serves all 8 cores, with per-core differences carried entirely
in the data (query permutation + mask tensor).

Precision: fp16 operands everywhere with fp32 PSUM accumulation
(~4e-4 output rel err); biases and normalization applied in fp32.
"""

import math
import sys

import numpy as np

for _p in ("/opt/trn_rl_repo", "/root/.axon_site/_ro/trn_rl_repo"):
    if _p not in sys.path:
        sys.path.insert(0, _p)

# ---- problem constants (hardcoded; kernel.py must be self-contained) ----
B, T, C = 4, 2048, 1024
P = 128                      # SBUF partitions
KC = C // P                  # 8 contraction chunks of 128
NQ = T // 2                  # queries per core
SW = 256                     # query-slot width (matmul free dim)
NSLOT = NQ // SW             # 4 slots per core
JBAR = (16, 12, 8, 4)        # tk-tile budget per slot (program constant)
A_FOLD = ((1792, 1280, 512, 0), (1536, 1024, 768, 256))
MASK_NJ = 4                  # mask applied to the last 4 tk-tiles of each slot
NEG = -30000.0
SCALE = 1.0 / math.sqrt(C)
NCORES = 8

_CACHE = {}


def _build_module():
    import concourse.bass as bass
    import concourse.bacc as bacc_mod
    import concourse.mybir as mybir
    import concourse.tile as tile
    from concourse.alu_op_type import AluOpType

    f32 = mybir.dt.float32
    f32r = mybir.dt.float32r
    f16 = mybir.dt.float16
    bf16 = mybir.dt.bfloat16
    AF = mybir.ActivationFunctionType

    nc = bacc_mod.Bacc("TRN2", target_bir_lowering=False, debug=False)

    qT = nc.dram_tensor("qt", [C, NQ], f16, kind="ExternalInput").ap()
    kT = nc.dram_tensor("kt", [C, T], f16, kind="ExternalInput").ap()
    vT = nc.dram_tensor("vt", [C, T], f16, kind="ExternalInput").ap()
    wqT = nc.dram_tensor("wqt", [C, C], f16, kind="ExternalInput").ap()
    wkT = nc.dram_tensor("wkt", [C, C], f16, kind="ExternalInput").ap()
    wvT = nc.dram_tensor("wvt", [C, C], f16, kind="ExternalInput").ap()
    wfT = nc.dram_tensor("wft", [C, C], f16, kind="ExternalInput").ap()
    bqs = nc.dram_tensor("bqs", [P, KC], f32, kind="ExternalInput").ap()
    bks = nc.dram_tensor("bks", [P, KC], f32, kind="ExternalInput").ap()
    bvb = nc.dram_tensor("bvb", [P, C], f32, kind="ExternalInput").ap()
    bfb = nc.dram_tensor("bfb", [P, C], f32, kind="ExternalInput").ap()
    maskt = nc.dram_tensor(
        "maskt", [P, NSLOT * MASK_NJ * SW], bf16, kind="ExternalInput"
    ).ap()
    out = nc.dram_tensor("out", [NQ, C], f32, kind="ExternalOutput").ap()

    with tile.TileContext(nc) as tc:
        with (
            tc.tile_pool(name="persist", bufs=1) as persist,
            tc.tile_pool(name="consts", bufs=1) as consts,
        ):
            # ---- persistent activations (live through phase 4) ----
            qpT = persist.tile([P, KC, NQ], f16, tag="qpT")    # 32KB/part
            kpT = persist.tile([P, KC, T], f16, tag="kpT")     # 64KB/part
            vp = persist.tile([P, T // P, C], f16, tag="vp")   # 32KB/part

            bqs_sb = consts.tile([P, KC], f32, tag="bqs")
            nc.sync.dma_start(out=bqs_sb, in_=bqs)
            bks_sb = consts.tile([P, KC], f32, tag="bks")
            nc.sync.dma_start(out=bks_sb, in_=bks)
            bvb_sb = consts.tile([P, C], f32, tag="bvb")
            nc.sync.dma_start(out=bvb_sb, in_=bvb)
            ones_f16 = consts.tile([P, 1], f16, tag="ones")
            nc.vector.memset(ones_f16, 1.0)
            ident11 = consts.tile([1, 1], f32, tag="ident11")
            nc.vector.memset(ident11, 1.0)
            # per-(slot, tq-tile) softmax reciprocal, filled during attention
            recip_sb = consts.tile([P, NSLOT * 2], f32, tag="recip")

            # ================= phase 1: kp^T = Wk-free k^T + bk ==============
            # kpT[p, m, t] = kp[t, m*128+p]
            with (
                tc.tile_pool(name="wst1", bufs=1) as wpool,
                tc.tile_pool(name="xst1", bufs=16) as xpool,
                tc.tile_pool(name="pps1", bufs=4, space="PSUM") as ppsum,
            ):
                wk_sb = []
                for kk in range(KC):
                    wt = wpool.tile([P, C], f16, tag=f"wk{kk}")
                    nc.sync.dma_start(out=wt, in_=wkT[kk * P:(kk + 1) * P, :])
                    wk_sb.append(wt)
                for n in range(T // SW):  # 8 ranges of 256
                    xs = []
                    for kk in range(KC):
                        xt = xpool.tile([P, SW], f16, tag="xk")
                        nc.sync.dma_start(
                            out=xt,
                            in_=kT[kk * P:(kk + 1) * P, n * SW:(n + 1) * SW],
                        )
                        xs.append(xt)
                    for m in range(KC):
                        ps = ppsum.tile([P, SW], f32, tag="ps")
                        for kk in range(KC):
                            nc.tensor.matmul(
                                ps,
                                wk_sb[kk][:, m * P:(m + 1) * P],
                                xs[kk],
                                start=(kk == 0),
                                stop=(kk == KC - 1),
                            )
                        nc.scalar.activation(
                            kpT[:, m, n * SW:(n + 1) * SW],
                            ps,
                            AF.Identity,
                            bias=bks_sb[:, m:m + 1],
                        )

            # ================= phase 2: qp^T = (Wq-free q^T + bq) * scale ====
            with (
                tc.tile_pool(name="wst2", bufs=1) as wpool,
                tc.tile_pool(name="xst2", bufs=16) as xpool,
                tc.tile_pool(name="pps2", bufs=4, space="PSUM") as ppsum,
            ):
                wq_sb = []
                for kk in range(KC):
                    wt = wpool.tile([P, C], f16, tag=f"wq{kk}")
                    nc.sync.dma_start(out=wt, in_=wqT[kk * P:(kk + 1) * P, :])
                    wq_sb.append(wt)
                for n in range(NQ // SW):  # 4 ranges of 256
                    xs = []
                    for kk in range(KC):
                        xt = xpool.tile([P, SW], f16, tag="xq")
                        nc.sync.dma_start(
                            out=xt,
                            in_=qT[kk * P:(kk + 1) * P, n * SW:(n + 1) * SW],
                        )
                        xs.append(xt)
                    for m in range(KC):
                        ps = ppsum.tile([P, SW], f32, tag="ps")
                        for kk in range(KC):
                            nc.tensor.matmul(
                                ps,
                                wq_sb[kk][:, m * P:(m + 1) * P],
                                xs[kk],
                                start=(kk == 0),
                                stop=(kk == KC - 1),
                            )
                        nc.scalar.activation(
                            qpT[:, m, n * SW:(n + 1) * SW],
                            ps,
                            AF.Identity,
                            bias=bqs_sb[:, m:m + 1],
                            scale=SCALE,
                        )

            # ================= phase 3: vp = v Wv^T + bv (tk on partitions) ==
            with (
                tc.tile_pool(name="wst3", bufs=1) as wpool,
                tc.tile_pool(name="xst3", bufs=24) as xpool,
                tc.tile_pool(name="pps3", bufs=4, space="PSUM") as ppsum,
            ):
                wv_sb = []
                for kk in range(KC):
                    wt = wpool.tile([P, C], f16, tag=f"wv{kk}")
                    nc.sync.dma_start(out=wt, in_=wvT[kk * P:(kk + 1) * P, :])
                    wv_sb.append(wt)
                for m in range(T // P):  # 16 tk tiles
                    xs = []
                    for kk in range(KC):
                        xt = xpool.tile([P, P], f16, tag="xv")
                        nc.sync.dma_start(
                            out=xt,
                            in_=vT[kk * P:(kk + 1) * P, m * P:(m + 1) * P],
                        )
                        xs.append(xt)
                    for n in range(2):
                        ps = ppsum.tile([P, 512], f32, tag="ps")
                        for kk in range(KC):
                            nc.tensor.matmul(
                                ps,
                                xs[kk],
                                wv_sb[kk][:, n * 512:(n + 1) * 512],
                                start=(kk == 0),
                                stop=(kk == KC - 1),
                            )
                        nc.vector.tensor_tensor(
                            vp[:, m, n * 512:(n + 1) * 512],
                            ps,
                            bvb_sb[:, n * 512:(n + 1) * 512],
                            op=AluOpType.add,
                        )

            # ============ phases 4+5 share the attn_out pool =================
            with tc.tile_pool(name="aop", bufs=4) as aopool:
                attn_out = []
                # ------------- phase 4: attention ----------------------------
                with (
                    tc.tile_pool(name="msk", bufs=1) as mpool,
                    tc.tile_pool(name="expp", bufs=1) as epool,
                    tc.tile_pool(name="sps", bufs=2, space="PSUM") as spsum,
                    tc.tile_pool(name="dps", bufs=2, space="PSUM") as dpsum,
                    tc.tile_pool(name="vps", bufs=4, space="PSUM") as vpsum,
                ):
                    mask_sb = mpool.tile(
                        [P, NSLOT * MASK_NJ * SW], bf16, tag="mask"
                    )
                    nc.sync.dma_start(out=mask_sb, in_=maskt)
                    for s in range(NSLOT):
                        jbar = JBAR[s]
                        ao = aopool.tile([P, KC, SW], f16, tag="ao")
                        attn_out.append(ao)
                        expT = epool.tile([P, 16, SW], f16, tag="expT")
                        denom_ps = dpsum.tile([1, SW], f32, tag="denom")
                        for j in range(jbar):
                            sc_ps = spsum.tile([P, SW], f32, tag="sc")
                            for kk in range(KC):
                                nc.tensor.matmul(
                                    sc_ps,
                                    kpT[:, kk, j * P:(j + 1) * P],
                                    qpT[:, kk, s * SW:(s + 1) * SW],
                                    start=(kk == 0),
                                    stop=(kk == KC - 1),
                                )
                            jj = j - (jbar - MASK_NJ)
                            if jj >= 0:
                                off = (s * MASK_NJ + jj) * SW
                                nc.vector.tensor_tensor(
                                    sc_ps,
                                    sc_ps,
                                    mask_sb[:, off:off + SW],
                                    op=AluOpType.add,
                                )
                            nc.scalar.activation(expT[:, j, :], sc_ps, AF.Exp)
                            nc.tensor.matmul(
                                denom_ps,
                                ones_f16,
                                expT[:, j, :],
                                start=(j == 0),
                                stop=(j == jbar - 1),
                            )
                        # transpose the [1, SW] denominator row into two
                        # [128, 1] per-partition columns (PE transpose), recip
                        drow = epool.tile([1, SW], f32, tag="drow", bufs=2)
                        nc.vector.tensor_copy(drow, denom_ps)
                        for t in range(2):
                            cidx = s * 2 + t
                            dT_ps = spsum.tile([P, 1], f32, tag="sc")
                            nc.tensor.transpose(
                                dT_ps,
                                drow[0:1, t * P:(t + 1) * P],
                                ident11,
                            )
                            nc.vector.reciprocal(
                                recip_sb[:, cidx:cidx + 1],
                                dT_ps,
                            )
                        # PV: ao[d, tq] += vp-free[tk, d] x exp^T[tk, tq]
                        for g in range(2):
                            for mm in range(4):
                                m = g * 4 + mm
                                pv_ps = vpsum.tile([P, SW], f32, tag="pv")
                                for j in range(jbar):
                                    nc.tensor.matmul(
                                        pv_ps,
                                        vp[:, j, m * P:(m + 1) * P],
                                        expT[:, j, :],
                                        start=(j == 0),
                                        stop=(j == jbar - 1),
                                    )
                                nc.vector.tensor_copy(ao[:, m, :], pv_ps)

                # ------------- phase 5: out = norm(ao) Wff^T + bff -----------
                with (
                    tc.tile_pool(name="wst5", bufs=12) as wpool,
                    tc.tile_pool(name="bfp", bufs=1) as bfpool,
                    tc.tile_pool(name="ost", bufs=4) as opool,
                    tc.tile_pool(name="fps", bufs=4, space="PSUM") as fpsum,
                ):
                    bfb_sb = bfpool.tile([P, C], f32, tag="bfb")
                    nc.sync.dma_start(out=bfb_sb, in_=bfb)
                    for h in range(2):  # cout halves; Wff streamed per half
                        wf_h = []
                        for kk in range(KC):
                            wt = wpool.tile([P, 512], f16, tag="wf")
                            nc.sync.dma_start(
                                out=wt,
                                in_=wfT[kk * P:(kk + 1) * P,
                                        h * 512:(h + 1) * 512],
                            )
                            wf_h.append(wt)
                        for s in range(NSLOT):
                            for t in range(2):
                                row = s * 2 + t
                                ps = fpsum.tile([P, 512], f32, tag="ff")
                                for kk in range(KC):
                                    nc.tensor.matmul(
                                        ps,
                                        attn_out[s][:, kk,
                                                      t * P:(t + 1) * P],
                                        wf_h[kk],
                                        start=(kk == 0),
                                        stop=(kk == KC - 1),
                                    )
                                st = opool.tile([P, 512], f32, tag="ost")
                                nc.vector.scalar_tensor_tensor(
                                    st,
                                    ps,
                                    recip_sb[:, row:row + 1],
                                    bfb_sb[:, h * 512:(h + 1) * 512],
                                    op0=AluOpType.mult,
                                    op1=AluOpType.add,
                                )
                                nc.sync.dma_start(
                                    out=out[row * P:(row + 1) * P,
                                            h * 512:(h + 1) * 512],
                                    in_=st,
                                )
    nc.compile()
    return nc


def _get_module():
    if "nc" not in _CACHE:
        _CACHE["nc"] = _build_module()
    return _CACHE["nc"]


def _host_prep(q, k, v, Wq, bq, Wk, bk, Wv, bv, Wff, bff):
    """Build the 8 per-core input maps."""
    import ml_dtypes

    f32 = np.float32
    q, k, v = (np.asarray(x, f32) for x in (q, k, v))
    Wq, Wk, Wv, Wff = (np.asarray(x, f32) for x in (Wq, Wk, Wv, Wff))
    bq, bk, bv, bff = (np.asarray(x, f32) for x in (bq, bk, bv, bff))

    f16 = np.float16
    wqT = np.ascontiguousarray(Wq.T.astype(f16))
    wkT = np.ascontiguousarray(Wk.T.astype(f16))
    wvT = np.ascontiguousarray(Wv.T.astype(f16))
    wfT = np.ascontiguousarray(Wff.T.astype(f16))
    bqs = np.ascontiguousarray((bq * SCALE).reshape(KC, P).T)
    bks = np.ascontiguousarray(bk.reshape(KC, P).T)
    bvb = np.ascontiguousarray(np.broadcast_to(bv, (P, C)))
    bfb = np.ascontiguousarray(np.broadcast_to(bff, (P, C)))

    kT = [np.ascontiguousarray(k[b].T.astype(f16)) for b in range(B)]
    vT = [np.ascontiguousarray(v[b].T.astype(f16)) for b in range(B)]

    # per-fold causal masks: 0 where tk <= tq else NEG
    masks = []
    pp = np.arange(P)[:, None]
    ff = np.arange(SW)[None, :]
    for f in range(2):
        m = np.zeros((P, NSLOT * MASK_NJ * SW), f32)
        for s in range(NSLOT):
            a = A_FOLD[f][s]
            for jj in range(MASK_NJ):
                j = JBAR[s] - MASK_NJ + jj
                blk = np.where(j * P + pp <= a + ff, 0.0, NEG).astype(f32)
                off = (s * MASK_NJ + jj) * SW
                m[:, off:off + SW] = blk
        masks.append(m.astype(ml_dtypes.bfloat16))

    in_maps = []
    for c in range(NCORES):
        b, f = c >> 1, c & 1
        qrows = np.concatenate(
            [q[b, a:a + SW, :] for a in A_FOLD[f]], axis=0
        )  # [NQ, C]
        qT_c = np.ascontiguousarray(qrows.T.astype(f16))
        in_maps.append(
            {
                "qt": qT_c,
                "kt": kT[b],
                "vt": vT[b],
                "wqt": wqT,
                "wkt": wkT,
                "wvt": wvT,
                "wft": wfT,
                "bqs": bqs,
                "bks": bks,
                "bvb": bvb,
                "bfb": bfb,
                "maskt": masks[f],
            }
        )
    return in_maps


def _assemble(results):
    out = np.empty((B, T, C), np.float32)
    for c in range(NCORES):
        b, f = c >> 1, c & 1
        res = results[c]["out"]
        for s in range(NSLOT):
            a = A_FOLD[f][s]
            out[b, a:a + SW, :] = res[s * SW:(s + 1) * SW, :]
    return out


def _run(inputs, trace=False, tmpdir=None):
    from concourse.bass_utils import run_bass_kernel_spmd

    nc = _get_module()
    in_maps = _host_prep(**inputs)
    res = run_bass_kernel_spmd(
        nc, in_maps, core_ids=list(range(NCORES)), trace=trace, tmpdir=tmpdir
    )
    return _assemble(res.results), res


def kernel(**inputs):
    out, _ = _run(inputs, trace=False)
    return out
